# revision 11
# baseline (speedup 1.0000x reference)
"""Trainium2 Bass kernel for nn_AlignmentEncoder.

Data-parallel over batch: 16 batches -> 8 cores x 2 batches.

Per core, per batch b:
  key path:   keys (512,256) -> keysT (256,512) via PE transpose ->
              conv k3 256->512 (PE) + relu -> conv k1 512->256 (PE) -> keT
              k2 = sum_c keT^2 (ACT square is avoided; DVE mult + PE ones-reduce)
              c2row = -TEMP * k2  (per-t2 row vector)
  query path: queries (80,2048) loads naturally as qT; conv chain on PE,
              epilogues (bias+relu) on DVE; qw3/qb3 pre-scaled by 2*TEMP so
              z = 2T*qk - T*k2 comes straight out of PSUM.
  scores, per t1-tile i (16 of 128 rows):
              z psum = qeT_i^T keT + ones x c2row           (PE)
              pm psum = priorT block (4x PE transpose)
              logP = Ln(pm + 1e-8)                          (ACT, f32)
              e1, sum1 = Exp(z), accum                      (ACT)
              lse = Ln(sum1)  [batched over 4 tiles]        (ACT)
              lp = (z - lse) + logP  -> attn_logprob out    (DVE stt, f32)
              e2 = Exp(lp)                                  (ACT, bf16)
              e2m, sum2 = e2 * m01rep, accum                (DVE ttr)
              attn = e2m * (1/sum2) -> attn out             (GPSIMD ts, f32)

log_softmax identities used: q2 term cancels entirely; no max-subtraction is
needed because z = 2T*qk - T*k2 is confined to a tiny range (|z| < ~1).
"""

import numpy as np

import concourse.bass as bass
import concourse.tile as tile
from concourse import bacc, mybir
from concourse.masks import make_identity

F32 = mybir.dt.float32
BF16 = mybir.dt.bfloat16
AF = mybir.ActivationFunctionType
OP = mybir.AluOpType

B, T1, T2 = 16, 2048, 512
N_MEL, N_TEXT, N_ATT = 80, 256, 256
TEMP = 0.0005
NCORES = 8
PB = B // NCORES  # batches per core
NT1 = T1 // 128   # t1 tiles per batch
EPS = 1e-8


def build_nc(repeat: int = 1):
    nc = bacc.Bacc("TRN2", target_bir_lowering=False, debug=False,
                   enable_asserts=False)

    # ---- per-core DRAM I/O ----
    d_q = nc.dram_tensor("queries", [PB, N_MEL, T1], F32, kind="ExternalInput").ap()
    d_k = nc.dram_tensor("keys", [PB, T2, N_TEXT], F32, kind="ExternalInput").ap()
    d_m01 = nc.dram_tensor("m01row", [PB, T2], F32, kind="ExternalInput").ap()
    d_pr = nc.dram_tensor("prior", [PB, T2, T1], F32, kind="ExternalInput").ap()
    d_kw1 = nc.dram_tensor("kw1", [3, N_TEXT, 2 * N_TEXT], F32, kind="ExternalInput").ap()
    d_kb1 = nc.dram_tensor("kb1", [2 * N_TEXT], F32, kind="ExternalInput").ap()
    d_kw2 = nc.dram_tensor("kw2", [2 * N_TEXT, N_ATT], F32, kind="ExternalInput").ap()
    d_kb2 = nc.dram_tensor("kb2", [N_ATT], F32, kind="ExternalInput").ap()
    d_qw1 = nc.dram_tensor("qw1", [3, N_MEL, 2 * N_MEL], F32, kind="ExternalInput").ap()
    d_qb1 = nc.dram_tensor("qb1", [2 * N_MEL], F32, kind="ExternalInput").ap()
    d_qw2 = nc.dram_tensor("qw2", [2 * N_MEL, N_MEL], F32, kind="ExternalInput").ap()
    d_qb2 = nc.dram_tensor("qb2", [N_MEL], F32, kind="ExternalInput").ap()
    d_qw3 = nc.dram_tensor("qw3", [N_MEL, N_ATT], F32, kind="ExternalInput").ap()
    d_qb3 = nc.dram_tensor("qb3", [N_ATT], F32, kind="ExternalInput").ap()
    d_attn = nc.dram_tensor("attn", [PB, 1, T1, T2], F32, kind="ExternalOutput").ap()
    d_lp = nc.dram_tensor("attn_logprob", [PB, 1, T1, T2], F32, kind="ExternalOutput").ap()

    with tile.TileContext(nc) as tc:
        _body(tc, repeat,
              d_q, d_k, d_m01, d_pr,
              d_kw1, d_kb1, d_kw2, d_kb2,
              d_qw1, d_qb1, d_qw2, d_qb2, d_qw3, d_qb3,
              d_attn, d_lp)
    nc.compile()
    return nc


def _body(tc, repeat, d_q, d_k, d_m01, d_pr, d_kw1, d_kb1, d_kw2, d_kb2,
          d_qw1, d_qb1, d_qw2, d_qb2, d_qw3, d_qb3, d_attn, d_lp):
    nc = tc.nc
    from contextlib import ExitStack
    ctx = ExitStack()
    with ctx:
        const = ctx.enter_context(tc.tile_pool(name="const", bufs=1))
        wpool = ctx.enter_context(tc.tile_pool(name="wpool", bufs=1))
        kpool = ctx.enter_context(tc.tile_pool(name="kpool", bufs=2))
        qpool = ctx.enter_context(tc.tile_pool(name="qpool", bufs=1))
        qepool = ctx.enter_context(tc.tile_pool(name="qepool", bufs=2))
        prpool = ctx.enter_context(tc.tile_pool(name="prpool", bufs=8))
        spool = ctx.enter_context(tc.tile_pool(name="spool", bufs=3))
        smallp = ctx.enter_context(tc.tile_pool(name="smallp", bufs=4))
        stgpool = ctx.enter_context(tc.tile_pool(name="stgpool", bufs=2))
        ps_z = ctx.enter_context(tc.tile_pool(name="ps_z", bufs=2, space="PSUM"))
        ps_pm = ctx.enter_context(tc.tile_pool(name="ps_pm", bufs=2, space="PSUM"))
        ps_cv = ctx.enter_context(tc.tile_pool(name="ps_cv", bufs=2, space="PSUM"))
        # all small PSUM tensors share one 2-slot tag (each <= 1 bank)
        ps_sm = ctx.enter_context(tc.tile_pool(name="ps_sm", bufs=2, space="PSUM"))

        def emit(it):
            # ---- constants ----
            ident_f = const.tile([128, 128], F32, name=f"ident_f{it}")
            make_identity(nc, ident_f[:])
            ident_b = const.tile([128, 128], BF16, name=f"ident_b{it}")
            make_identity(nc, ident_b[:])
            ones_row = const.tile([1, 128], BF16, name=f"ones_row{it}")
            nc.gpsimd.memset(ones_row[:], 1.0)
            ones_col = const.tile([128, 1], BF16, name=f"ones_col{it}")
            nc.gpsimd.memset(ones_col[:], 1.0)
            eps_col = const.tile([128, 1], F32, name=f"eps_col{it}")
            nc.gpsimd.memset(eps_col[:], EPS)

            # ---- weights (cast to bf16 during DMA on the SWDGE path) ----
            kw1_sb = wpool.tile([128, 3, 2, 2 * N_TEXT], BF16, name=f"kw1_sb{it}")
            nc.gpsimd.dma_start(
                out=kw1_sb[:],
                in_=d_kw1.rearrange("dt (ci p) o -> p dt ci o", p=128))
            kw2_sb = wpool.tile([128, 4, N_ATT], BF16, name=f"kw2_sb{it}")
            nc.gpsimd.dma_start(
                out=kw2_sb[:],
                in_=d_kw2.rearrange("(ci p) o -> p ci o", p=128))
            qw1_sb = wpool.tile([N_MEL, 3, 2 * N_MEL], BF16, name=f"qw1_sb{it}")
            nc.gpsimd.dma_start(
                out=qw1_sb[:], in_=d_qw1.rearrange("dt ci o -> ci dt o"))
            qw2a_sb = wpool.tile([128, N_MEL], BF16, name=f"qw2a_sb{it}")
            nc.gpsimd.dma_start(out=qw2a_sb[:], in_=d_qw2[0:128, :])
            qw2b_sb = wpool.tile([32, N_MEL], BF16, name=f"qw2b_sb{it}")
            nc.gpsimd.dma_start(out=qw2b_sb[:], in_=d_qw2[128:160, :])
            qw3_f = wpool.tile([N_MEL, N_ATT], F32, name=f"qw3_f{it}")
            nc.sync.dma_start(out=qw3_f[:], in_=d_qw3[:])
            qw3_sb = wpool.tile([N_MEL, N_ATT], BF16, name=f"qw3_sb{it}")
            nc.vector.tensor_scalar_mul(qw3_sb[:], qw3_f[:], 2.0 * TEMP)

            # biases as [128, ncols] column stacks
            kb1_sb = wpool.tile([128, 4], F32, name=f"kb1_sb{it}")
            nc.sync.dma_start(out=kb1_sb[:], in_=d_kb1.rearrange("(j p) -> p j", p=128))
            kb2_sb = wpool.tile([128, 2], F32, name=f"kb2_sb{it}")
            nc.sync.dma_start(out=kb2_sb[:], in_=d_kb2.rearrange("(j p) -> p j", p=128))
            qb1_sb = wpool.tile([128, 2], F32, name=f"qb1_sb{it}")
            nc.gpsimd.memset(qb1_sb[:], 0.0)
            nc.sync.dma_start(out=qb1_sb[0:128, 0:1], in_=d_qb1[0:128].rearrange("(p o) -> p o", o=1))
            nc.sync.dma_start(out=qb1_sb[0:32, 1:2], in_=d_qb1[128:160].rearrange("(p o) -> p o", o=1))
            qb2_sb = wpool.tile([N_MEL, 1], F32, name=f"qb2_sb{it}")
            nc.sync.dma_start(out=qb2_sb[:], in_=d_qb2.rearrange("(p o) -> p o", o=1))
            qb3_f = wpool.tile([128, 2], F32, name=f"qb3_f{it}")
            nc.sync.dma_start(out=qb3_f[:], in_=d_qb3.rearrange("(j p) -> p j", p=128))
            qb3_sb = wpool.tile([128, 2], F32, name=f"qb3_sb{it}")
            nc.vector.tensor_scalar_mul(qb3_sb[:], qb3_f[:], 2.0 * TEMP)

            for b in range(PB):
                # ================= key path =================
                keys_nat = kpool.tile([128, 4, N_TEXT], BF16, tag="keys_nat")
                nc.gpsimd.dma_start(
                    out=keys_nat[:],
                    in_=d_k[b].rearrange("(j p) c -> p j c", p=128))
                # keysT: [c, t2] with zero-padded t2 edges, 2 c-tiles
                keysT = [kpool.tile([128, T2 + 2], BF16, tag=f"keysT{ci}", name=f"keysT{ci}")
                         for ci in range(2)]
                for ci in range(2):
                    nc.gpsimd.memset(keysT[ci][:, 0:1], 0.0)
                    nc.gpsimd.memset(keysT[ci][:, T2 + 1:T2 + 2], 0.0)
                for j in range(4):
                    for ci in range(2):
                        pst = ps_sm.tile([128, 128], BF16, tag="sm", name="pst")
                        nc.tensor.transpose(pst[:], keys_nat[:, j, ci * 128:(ci + 1) * 128],
                                            ident_b[:])
                        nc.vector.tensor_copy(keysT[ci][:, 1 + j * 128:1 + (j + 1) * 128],
                                              pst[:])
                # kconv1 (k=3, 256->512) + relu
                ke1T = [kpool.tile([128, T2], BF16, tag=f"ke1T{j}", name=f"ke1T{j}") for j in range(4)]
                for j in range(4):
                    pcv = ps_cv.tile([128, T2], F32, tag="pcv")
                    first = True
                    for dt in range(3):
                        for ci in range(2):
                            nc.tensor.matmul(
                                pcv[:], kw1_sb[:, dt, ci, j * 128:(j + 1) * 128],
                                keysT[ci][:, dt:dt + T2],
                                start=first, stop=(dt == 2 and ci == 1))
                            first = False
                    nc.scalar.activation(ke1T[j][:], pcv[:], AF.Relu,
                                         bias=kb1_sb[:, j:j + 1])
                # kconv2 (k=1, 512->256)
                keT = [kpool.tile([128, T2], BF16, tag=f"keT{j2}", name=f"keT{j2}") for j2 in range(2)]
                for j2 in range(2):
                    pcv = ps_cv.tile([128, T2], F32, tag="pcv")
                    for ci1 in range(4):
                        nc.tensor.matmul(pcv[:], kw2_sb[:, ci1, j2 * 128:(j2 + 1) * 128],
                                         ke1T[ci1][:],
                                         start=(ci1 == 0), stop=(ci1 == 3))
                    nc.scalar.activation(keT[j2][:], pcv[:], AF.Identity,
                                         bias=kb2_sb[:, j2:j2 + 1])
                # k2 = sum_c keT^2 ; c2row = -TEMP * k2
                sqk = [kpool.tile([128, T2], BF16, tag=f"sqk{j2}", name=f"sqk{j2}") for j2 in range(2)]
                for j2 in range(2):
                    nc.vector.tensor_mul(sqk[j2][:], keT[j2][:], keT[j2][:])
                pk2 = ps_sm.tile([1, T2], F32, tag="sm", name="pk2")
                for j2 in range(2):
                    nc.tensor.matmul(pk2[:], ones_col[:], sqk[j2][:],
                                     start=(j2 == 0), stop=(j2 == 1))
                c2row = smallp.tile([1, T2], BF16, tag="c2row")
                nc.scalar.activation(c2row[:], pk2[:], AF.Copy, scale=-TEMP)

                # m01rep: [128, T2] bf16 broadcast of the valid-mask row
                m01_b = smallp.tile([1, T2], BF16, tag="m01_b")
                nc.gpsimd.dma_start(out=m01_b[:], in_=d_m01[b].rearrange("(o t) -> o t", o=1))
                pmr = ps_sm.tile([128, T2], F32, tag="sm", name="pmr")
                nc.tensor.matmul(pmr[:], ones_row[:], m01_b[:], start=True, stop=True)
                m01rep = kpool.tile([128, T2], BF16, tag="m01rep")
                nc.scalar.activation(m01rep[:], pmr[:], AF.Copy)

                # ================= query path =================
                qT = qpool.tile([N_MEL, T1 + 2], BF16, tag="qT")
                nc.gpsimd.memset(qT[:, 0:1], 0.0)
                nc.gpsimd.memset(qT[:, T1 + 1:T1 + 2], 0.0)
                nc.gpsimd.dma_start(out=qT[:, 1:T1 + 1], in_=d_q[b])
                # qconv1 (k=3, 80->160) + relu: o-tiles [128, 32]
                qe1a = qpool.tile([128, T1], BF16, tag="qe1a")
                qe1b = qpool.tile([32, T1], BF16, tag="qe1b")
                for n in range(4):
                    for (oi, (qe1, o0, ow)) in enumerate(
                            [(qe1a, 0, 128), (qe1b, 128, 32)]):
                        pcv = ps_cv.tile([128, T2], F32, tag="pcv")
                        for dt in range(3):
                            nc.tensor.matmul(
                                pcv[0:ow, :], qw1_sb[:, dt, o0:o0 + ow],
                                qT[:, dt + n * T2:dt + (n + 1) * T2],
                                start=(dt == 0), stop=(dt == 2))
                        nc.vector.tensor_scalar(
                            qe1[:, n * T2:(n + 1) * T2], pcv[0:ow, :],
                            qb1_sb[0:ow, oi:oi + 1], 0.0, OP.add, OP.max)
                # qconv2 (k=1, 160->80) + relu
                qe2 = qpool.tile([N_MEL, T1], BF16, tag="qe2")
                for n in range(4):
                    pcv = ps_cv.tile([128, T2], F32, tag="pcv")
                    nc.tensor.matmul(pcv[0:N_MEL, :], qw2a_sb[:],
                                     qe1a[:, n * T2:(n + 1) * T2],
                                     start=True, stop=False)
                    nc.tensor.matmul(pcv[0:N_MEL, :], qw2b_sb[:],
                                     qe1b[:, n * T2:(n + 1) * T2],
                                     start=False, stop=True)
                    nc.vector.tensor_scalar(qe2[:, n * T2:(n + 1) * T2],
                                            pcv[0:N_MEL, :], qb2_sb[:],
                                            0.0, OP.add, OP.max)
                # qconv3 (k=1, 80->256), scaled by 2*TEMP
                qeT = [qepool.tile([128, T1], BF16, tag=f"qeT{o}", name=f"qeT{o}") for o in range(2)]
                for o in range(2):
                    for n in range(4):
                        pcv = ps_cv.tile([128, T2], F32, tag="pcv")
                        nc.tensor.matmul(pcv[:], qw3_sb[:, o * 128:(o + 1) * 128],
                                         qe2[:, n * T2:(n + 1) * T2],
                                         start=True, stop=True)
                        nc.vector.tensor_scalar(qeT[o][:, n * T2:(n + 1) * T2],
                                                pcv[:], qb3_sb[:, o:o + 1],
                                                None, OP.add)

                # ================= prior loads (2 halves of 4 tiles) =================
                prh = {}
                for h in range(2):
                    for j in range(4):
                        prt = prpool.tile([128, T1 // 2], F32, tag="prt")
                        nc.sync.dma_start(
                            out=prt[:],
                            in_=d_pr[b, j * 128:(j + 1) * 128,
                                     h * (T1 // 2):(h + 1) * (T1 // 2)])
                        prh[(h, j)] = prt

                # ================= scores =================
                for i in range(NT1):
                    h, hoff = (0, 0) if i < 8 else (1, T1 // 2)
                    iq = i % 4
                    # z = 2T*qk - T*k2
                    pz = ps_z.tile([128, T2], F32, tag="pz")
                    nc.tensor.matmul(pz[:], qeT[0][:, i * 128:(i + 1) * 128],
                                     keT[0][:], start=True, stop=False)
                    nc.tensor.matmul(pz[:], qeT[1][:, i * 128:(i + 1) * 128],
                                     keT[1][:], start=False, stop=False)
                    nc.tensor.matmul(pz[:], ones_row[:], c2row[:],
                                     start=False, stop=True)
                    # priorT block via 4 PE transposes
                    ppm = ps_pm.tile([128, T2], F32, tag="ppm")
                    for j in range(4):
                        nc.tensor.transpose(
                            ppm[:, j * 128:(j + 1) * 128],
                            prh[(h, j)][:, i * 128 - hoff:(i + 1) * 128 - hoff],
                            ident_f[:])
                    logP = spool.tile([128, T2], F32, tag="logP")
                    nc.scalar.activation(logP[:], ppm[:], AF.Ln, bias=eps_col[:])
                    # lse = Ln(sum(Exp(z))): no max-subtraction needed, |z| tiny
                    sum1 = smallp.tile([128, 1], F32, tag="sum1")
                    e1 = spool.tile([128, T2], BF16, tag="e1")
                    nc.scalar.activation(e1[:], pz[:], AF.Exp, accum_out=sum1[:])
                    lse = smallp.tile([128, 1], F32, tag="lse")
                    nc.scalar.activation(lse[:], sum1[:], AF.Ln)

                    # stage buffers for 4 tiles then one 1MB DMA out
                    if iq == 0:
                        lp4 = stgpool.tile([128, 4, T2], F32, tag="lp4")
                        at4 = stgpool.tile([128, 4, T2], F32, tag="at4")
                    nc.vector.scalar_tensor_tensor(
                        lp4[:, iq, :], pz[:], lse[:], logP[:],
                        OP.subtract, OP.add)
                    e2 = spool.tile([128, T2], BF16, tag="e2")
                    nc.scalar.activation(e2[:], lp4[:, iq, :], AF.Exp)
                    e2m = spool.tile([128, T2], BF16, tag="e2m")
                    sum2 = smallp.tile([128, 1], F32, tag="sum2")
                    nc.vector.scalar_tensor_tensor(
                        e2m[:], e2[:], 1.0, m01rep[:],
                        OP.mult, OP.mult, accum_out=sum2[:])
                    r2 = smallp.tile([128, 1], F32, tag="r2")
                    nc.vector.reciprocal(r2[:], sum2[:])
                    nc.gpsimd.tensor_scalar(at4[:, iq, :], e2m[:], r2[:],
                                            None, OP.mult)
                    if iq == 3:
                        i0 = i - 3
                        nc.sync.dma_start(
                            out=d_lp[b, 0, i0 * 128:(i0 + 4) * 128, :]
                            .rearrange("(g p) t -> p g t", p=128),
                            in_=lp4[:])
                        nc.sync.dma_start(
                            out=d_attn[b, 0, i0 * 128:(i0 + 4) * 128, :]
                            .rearrange("(g p) t -> p g t", p=128),
                            in_=at4[:])

        if repeat == 1:
            emit(0)
        else:
            with tc.For_i(0, repeat, 1):
                emit(0)


_CACHE = {}


def _get_nc(repeat: int = 1):
    if repeat not in _CACHE:
        _CACHE[repeat] = build_nc(repeat)
    return _CACHE[repeat]


def make_in_maps(queries, keys, mask, attn_prior,
                 kw1, kb1, kw2, kb2, qw1, qb1, qw2, qb2, qw3, qb3):
    queries = np.ascontiguousarray(queries, dtype=np.float32)
    keys = np.ascontiguousarray(keys, dtype=np.float32)
    attn_prior = np.ascontiguousarray(attn_prior, dtype=np.float32)
    m01 = np.ascontiguousarray(1.0 - np.asarray(mask, dtype=np.float32))
    w = dict(
        kw1=np.ascontiguousarray(kw1, dtype=np.float32),
        kb1=np.ascontiguousarray(kb1, dtype=np.float32),
        kw2=np.ascontiguousarray(np.asarray(kw2, dtype=np.float32).reshape(2 * N_TEXT, N_ATT)),
        kb2=np.ascontiguousarray(kb2, dtype=np.float32),
        qw1=np.ascontiguousarray(qw1, dtype=np.float32),
        qb1=np.ascontiguousarray(qb1, dtype=np.float32),
        qw2=np.ascontiguousarray(np.asarray(qw2, dtype=np.float32).reshape(2 * N_MEL, N_MEL)),
        qb2=np.ascontiguousarray(qb2, dtype=np.float32),
        qw3=np.ascontiguousarray(np.asarray(qw3, dtype=np.float32).reshape(N_MEL, N_ATT)),
        qb3=np.ascontiguousarray(qb3, dtype=np.float32),
    )
    in_maps = []
    for c in range(NCORES):
        s = slice(c * PB, (c + 1) * PB)
        in_maps.append(dict(
            queries=queries[s], keys=keys[s], m01row=m01[s], prior=attn_prior[s],
            **w))
    return in_maps


def kernel(queries, keys, mask, attn_prior,
           kw1, kb1, kw2, kb2, qw1, qb1, qw2, qb2, qw3, qb3):
    from concourse import bass_utils
    nc = _get_nc(1)
    in_maps = make_in_maps(queries, keys, mask, attn_prior,
                           kw1, kb1, kw2, kb2, qw1, qb1, qw2, qb2, qw3, qb3)
    res = bass_utils.run_bass_kernel_spmd(nc, in_maps, core_ids=list(range(NCORES)))
    attn = np.concatenate([res.results[c]["attn"] for c in range(NCORES)], axis=0)
    lp = np.concatenate([res.results[c]["attn_logprob"] for c in range(NCORES)], axis=0)
    return attn, lp


# revision 13
# speedup vs baseline: 1.4377x; 1.4377x over previous
"""Trainium2 Bass kernel for nn_AlignmentEncoder.

Data-parallel over batch: 16 batches -> 8 cores x 2 batches.

Per core, per batch b:
  key path:   keys (512,256) -> keysT (256,512) via PE transpose ->
              conv k3 256->512 (PE) + relu -> conv k1 512->256 (PE) -> keT
              k2 = sum_c keT^2 (ACT square is avoided; DVE mult + PE ones-reduce)
              c2row = -TEMP * k2  (per-t2 row vector)
  query path: queries (80,2048) loads naturally as qT; conv chain on PE,
              epilogues (bias+relu) on DVE; qw3/qb3 pre-scaled by 2*TEMP so
              z = 2T*qk - T*k2 comes straight out of PSUM.
  scores, per t1-tile i (16 of 128 rows):
              z psum = qeT_i^T keT + ones x c2row           (PE)
              pm psum = priorT block (4x PE transpose)
              logP = Ln(pm + 1e-8)                          (ACT, f32)
              e1, sum1 = Exp(z), accum                      (ACT)
              lse = Ln(sum1)  [batched over 4 tiles]        (ACT)
              lp = (z - lse) + logP  -> attn_logprob out    (DVE stt, f32)
              e2 = Exp(lp)                                  (ACT, bf16)
              e2m, sum2 = e2 * m01rep, accum                (DVE ttr)
              attn = e2m * (1/sum2) -> attn out             (GPSIMD ts, f32)

log_softmax identities used: q2 term cancels entirely; no max-subtraction is
needed because z = 2T*qk - T*k2 is confined to a tiny range (|z| < ~1).
"""

import numpy as np

import concourse.bass as bass
import concourse.tile as tile
from concourse import bacc, mybir
from concourse.masks import make_identity

F32 = mybir.dt.float32
BF16 = mybir.dt.bfloat16
AF = mybir.ActivationFunctionType
OP = mybir.AluOpType

B, T1, T2 = 16, 2048, 512
N_MEL, N_TEXT, N_ATT = 80, 256, 256
TEMP = 0.0005
NCORES = 8
PB = B // NCORES  # batches per core
NT1 = T1 // 128   # t1 tiles per batch
EPS = 1e-8


def build_nc(repeat: int = 1):
    nc = bacc.Bacc("TRN2", target_bir_lowering=False, debug=False,
                   enable_asserts=False)

    # ---- per-core DRAM I/O ----
    d_q = nc.dram_tensor("queries", [PB, N_MEL, T1], F32, kind="ExternalInput").ap()
    d_k = nc.dram_tensor("keys", [PB, T2, N_TEXT], F32, kind="ExternalInput").ap()
    d_m01 = nc.dram_tensor("m01row", [PB, T2], F32, kind="ExternalInput").ap()
    d_pr = nc.dram_tensor("prior", [PB, T2, T1], F32, kind="ExternalInput").ap()
    d_kw1 = nc.dram_tensor("kw1", [3, N_TEXT, 2 * N_TEXT], F32, kind="ExternalInput").ap()
    d_kb1 = nc.dram_tensor("kb1", [2 * N_TEXT], F32, kind="ExternalInput").ap()
    d_kw2 = nc.dram_tensor("kw2", [2 * N_TEXT, N_ATT], F32, kind="ExternalInput").ap()
    d_kb2 = nc.dram_tensor("kb2", [N_ATT], F32, kind="ExternalInput").ap()
    d_qw1 = nc.dram_tensor("qw1", [3, N_MEL, 2 * N_MEL], F32, kind="ExternalInput").ap()
    d_qb1 = nc.dram_tensor("qb1", [2 * N_MEL], F32, kind="ExternalInput").ap()
    d_qw2 = nc.dram_tensor("qw2", [2 * N_MEL, N_MEL], F32, kind="ExternalInput").ap()
    d_qb2 = nc.dram_tensor("qb2", [N_MEL], F32, kind="ExternalInput").ap()
    d_qw3 = nc.dram_tensor("qw3", [N_MEL, N_ATT], F32, kind="ExternalInput").ap()
    d_qb3 = nc.dram_tensor("qb3", [N_ATT], F32, kind="ExternalInput").ap()
    d_attn = nc.dram_tensor("attn", [PB, 1, T1, T2], F32, kind="ExternalOutput").ap()
    d_lp = nc.dram_tensor("attn_logprob", [PB, 1, T1, T2], F32, kind="ExternalOutput").ap()

    with tile.TileContext(nc) as tc:
        _body(tc, repeat,
              d_q, d_k, d_m01, d_pr,
              d_kw1, d_kb1, d_kw2, d_kb2,
              d_qw1, d_qb1, d_qw2, d_qb2, d_qw3, d_qb3,
              d_attn, d_lp)
    nc.compile()
    return nc


def _body(tc, repeat, d_q, d_k, d_m01, d_pr, d_kw1, d_kb1, d_kw2, d_kb2,
          d_qw1, d_qb1, d_qw2, d_qb2, d_qw3, d_qb3, d_attn, d_lp):
    nc = tc.nc
    from contextlib import ExitStack
    ctx = ExitStack()
    with ctx:
        const = ctx.enter_context(tc.tile_pool(name="const", bufs=1))
        wpool = ctx.enter_context(tc.tile_pool(name="wpool", bufs=1))
        kpool = ctx.enter_context(tc.tile_pool(name="kpool", bufs=2))
        qpool = ctx.enter_context(tc.tile_pool(name="qpool", bufs=1))
        qepool = ctx.enter_context(tc.tile_pool(name="qepool", bufs=2))
        prpool = ctx.enter_context(tc.tile_pool(name="prpool", bufs=8))
        spool = ctx.enter_context(tc.tile_pool(name="spool", bufs=4))
        smallp = ctx.enter_context(tc.tile_pool(name="smallp", bufs=6))
        stgpool = ctx.enter_context(tc.tile_pool(name="stgpool", bufs=2))
        ps_z = ctx.enter_context(tc.tile_pool(name="ps_z", bufs=3, space="PSUM"))
        ps_pm = ctx.enter_context(tc.tile_pool(name="ps_pm", bufs=1, space="PSUM"))
        ps_cv = ctx.enter_context(tc.tile_pool(name="ps_cv", bufs=2, space="PSUM"))
        # all small PSUM tensors share one 2-slot tag (each <= 1 bank)
        ps_sm = ctx.enter_context(tc.tile_pool(name="ps_sm", bufs=2, space="PSUM"))

        def emit(it):
            # ---- constants ----
            ident_f = const.tile([128, 128], F32, name=f"ident_f{it}")
            make_identity(nc, ident_f[:])
            ident_b = const.tile([128, 128], BF16, name=f"ident_b{it}")
            make_identity(nc, ident_b[:])
            ones_row = const.tile([1, 128], BF16, name=f"ones_row{it}")
            nc.gpsimd.memset(ones_row[:], 1.0)
            ones_col = const.tile([128, 1], BF16, name=f"ones_col{it}")
            nc.gpsimd.memset(ones_col[:], 1.0)
            eps_col = const.tile([128, 1], F32, name=f"eps_col{it}")
            nc.gpsimd.memset(eps_col[:], EPS)

            # ---- weights (cast to bf16 during DMA on the SWDGE path) ----
            kw1_sb = wpool.tile([128, 3, 2, 2 * N_TEXT], BF16, name=f"kw1_sb{it}")
            nc.gpsimd.dma_start(
                out=kw1_sb[:],
                in_=d_kw1.rearrange("dt (ci p) o -> p dt ci o", p=128))
            kw2_sb = wpool.tile([128, 4, N_ATT], BF16, name=f"kw2_sb{it}")
            nc.gpsimd.dma_start(
                out=kw2_sb[:],
                in_=d_kw2.rearrange("(ci p) o -> p ci o", p=128))
            qw1_sb = wpool.tile([N_MEL, 3, 2 * N_MEL], BF16, name=f"qw1_sb{it}")
            nc.gpsimd.dma_start(
                out=qw1_sb[:], in_=d_qw1.rearrange("dt ci o -> ci dt o"))
            qw2a_sb = wpool.tile([128, N_MEL], BF16, name=f"qw2a_sb{it}")
            nc.gpsimd.dma_start(out=qw2a_sb[:], in_=d_qw2[0:128, :])
            qw2b_sb = wpool.tile([32, N_MEL], BF16, name=f"qw2b_sb{it}")
            nc.gpsimd.dma_start(out=qw2b_sb[:], in_=d_qw2[128:160, :])
            qw3_f = wpool.tile([N_MEL, N_ATT], F32, name=f"qw3_f{it}")
            nc.sync.dma_start(out=qw3_f[:], in_=d_qw3[:])
            qw3_sb = wpool.tile([N_MEL, N_ATT], BF16, name=f"qw3_sb{it}")
            nc.vector.tensor_scalar_mul(qw3_sb[:], qw3_f[:], 2.0 * TEMP)

            # biases as [128, ncols] column stacks
            kb1_sb = wpool.tile([128, 4], F32, name=f"kb1_sb{it}")
            nc.sync.dma_start(out=kb1_sb[:], in_=d_kb1.rearrange("(j p) -> p j", p=128))
            kb2_sb = wpool.tile([128, 2], F32, name=f"kb2_sb{it}")
            nc.sync.dma_start(out=kb2_sb[:], in_=d_kb2.rearrange("(j p) -> p j", p=128))
            qb1_sb = wpool.tile([128, 2], F32, name=f"qb1_sb{it}")
            nc.gpsimd.memset(qb1_sb[:], 0.0)
            nc.sync.dma_start(out=qb1_sb[0:128, 0:1], in_=d_qb1[0:128].rearrange("(p o) -> p o", o=1))
            nc.sync.dma_start(out=qb1_sb[0:32, 1:2], in_=d_qb1[128:160].rearrange("(p o) -> p o", o=1))
            qb2_sb = wpool.tile([N_MEL, 1], F32, name=f"qb2_sb{it}")
            nc.sync.dma_start(out=qb2_sb[:], in_=d_qb2.rearrange("(p o) -> p o", o=1))
            qb3_f = wpool.tile([128, 2], F32, name=f"qb3_f{it}")
            nc.sync.dma_start(out=qb3_f[:], in_=d_qb3.rearrange("(j p) -> p j", p=128))
            qb3_sb = wpool.tile([128, 2], F32, name=f"qb3_sb{it}")
            nc.vector.tensor_scalar_mul(qb3_sb[:], qb3_f[:], 2.0 * TEMP)

            for b in range(PB):
                # ================= key path =================
                keys_nat = kpool.tile([128, 4, N_TEXT], BF16, tag="keys_nat")
                nc.gpsimd.dma_start(
                    out=keys_nat[:],
                    in_=d_k[b].rearrange("(j p) c -> p j c", p=128))
                # keysT: [c, t2] with zero-padded t2 edges, 2 c-tiles
                keysT = [kpool.tile([128, T2 + 2], BF16, tag=f"keysT{ci}", name=f"keysT{ci}")
                         for ci in range(2)]
                for ci in range(2):
                    nc.gpsimd.memset(keysT[ci][:, 0:1], 0.0)
                    nc.gpsimd.memset(keysT[ci][:, T2 + 1:T2 + 2], 0.0)
                for j in range(4):
                    for ci in range(2):
                        pst = ps_sm.tile([128, 128], BF16, tag="sm", name="pst")
                        nc.tensor.transpose(pst[:], keys_nat[:, j, ci * 128:(ci + 1) * 128],
                                            ident_b[:])
                        nc.vector.tensor_copy(keysT[ci][:, 1 + j * 128:1 + (j + 1) * 128],
                                              pst[:])
                # kconv1 (k=3, 256->512) + relu
                ke1T = [kpool.tile([128, T2], BF16, tag=f"ke1T{j}", name=f"ke1T{j}") for j in range(4)]
                for j in range(4):
                    pcv = ps_cv.tile([128, T2], F32, tag="pcv")
                    first = True
                    for dt in range(3):
                        for ci in range(2):
                            nc.tensor.matmul(
                                pcv[:], kw1_sb[:, dt, ci, j * 128:(j + 1) * 128],
                                keysT[ci][:, dt:dt + T2],
                                start=first, stop=(dt == 2 and ci == 1))
                            first = False
                    nc.scalar.activation(ke1T[j][:], pcv[:], AF.Relu,
                                         bias=kb1_sb[:, j:j + 1])
                # kconv2 (k=1, 512->256)
                keT = [kpool.tile([128, T2], BF16, tag=f"keT{j2}", name=f"keT{j2}") for j2 in range(2)]
                for j2 in range(2):
                    pcv = ps_cv.tile([128, T2], F32, tag="pcv")
                    for ci1 in range(4):
                        nc.tensor.matmul(pcv[:], kw2_sb[:, ci1, j2 * 128:(j2 + 1) * 128],
                                         ke1T[ci1][:],
                                         start=(ci1 == 0), stop=(ci1 == 3))
                    nc.scalar.activation(keT[j2][:], pcv[:], AF.Identity,
                                         bias=kb2_sb[:, j2:j2 + 1])
                # k2 = sum_c keT^2 ; c2row = -TEMP * k2
                sqk = [kpool.tile([128, T2], BF16, tag=f"sqk{j2}", name=f"sqk{j2}") for j2 in range(2)]
                for j2 in range(2):
                    nc.vector.tensor_mul(sqk[j2][:], keT[j2][:], keT[j2][:])
                pk2 = ps_sm.tile([1, T2], F32, tag="sm", name="pk2")
                for j2 in range(2):
                    nc.tensor.matmul(pk2[:], ones_col[:], sqk[j2][:],
                                     start=(j2 == 0), stop=(j2 == 1))
                c2row = smallp.tile([1, T2], BF16, tag="c2row")
                nc.scalar.activation(c2row[:], pk2[:], AF.Copy, scale=-TEMP)

                # m01rep: [128, T2] bf16 broadcast of the valid-mask row
                m01_b = smallp.tile([1, T2], BF16, tag="m01_b")
                nc.gpsimd.dma_start(out=m01_b[:], in_=d_m01[b].rearrange("(o t) -> o t", o=1))
                pmr = ps_sm.tile([128, T2], F32, tag="sm", name="pmr")
                nc.tensor.matmul(pmr[:], ones_row[:], m01_b[:], start=True, stop=True)
                m01rep = kpool.tile([128, T2], BF16, tag="m01rep")
                nc.scalar.activation(m01rep[:], pmr[:], AF.Copy)

                # ================= query path =================
                qT = qpool.tile([N_MEL, T1 + 2], BF16, tag="qT")
                nc.gpsimd.memset(qT[:, 0:1], 0.0)
                nc.gpsimd.memset(qT[:, T1 + 1:T1 + 2], 0.0)
                nc.gpsimd.dma_start(out=qT[:, 1:T1 + 1], in_=d_q[b])
                # qconv1 (k=3, 80->160) + relu: o-tiles [128, 32]
                qe1a = qpool.tile([128, T1], BF16, tag="qe1a")
                qe1b = qpool.tile([32, T1], BF16, tag="qe1b")
                for n in range(4):
                    for (oi, (qe1, o0, ow)) in enumerate(
                            [(qe1a, 0, 128), (qe1b, 128, 32)]):
                        pcv = ps_cv.tile([128, T2], F32, tag="pcv")
                        for dt in range(3):
                            nc.tensor.matmul(
                                pcv[0:ow, :], qw1_sb[:, dt, o0:o0 + ow],
                                qT[:, dt + n * T2:dt + (n + 1) * T2],
                                start=(dt == 0), stop=(dt == 2))
                        nc.vector.tensor_scalar(
                            qe1[:, n * T2:(n + 1) * T2], pcv[0:ow, :],
                            qb1_sb[0:ow, oi:oi + 1], 0.0, OP.add, OP.max)
                # qconv2 (k=1, 160->80) + relu
                qe2 = qpool.tile([N_MEL, T1], BF16, tag="qe2")
                for n in range(4):
                    pcv = ps_cv.tile([128, T2], F32, tag="pcv")
                    nc.tensor.matmul(pcv[0:N_MEL, :], qw2a_sb[:],
                                     qe1a[:, n * T2:(n + 1) * T2],
                                     start=True, stop=False)
                    nc.tensor.matmul(pcv[0:N_MEL, :], qw2b_sb[:],
                                     qe1b[:, n * T2:(n + 1) * T2],
                                     start=False, stop=True)
                    nc.vector.tensor_scalar(qe2[:, n * T2:(n + 1) * T2],
                                            pcv[0:N_MEL, :], qb2_sb[:],
                                            0.0, OP.add, OP.max)
                # qconv3 (k=1, 80->256), scaled by 2*TEMP
                qeT = [qepool.tile([128, T1], BF16, tag=f"qeT{o}", name=f"qeT{o}") for o in range(2)]
                for o in range(2):
                    for n in range(4):
                        pcv = ps_cv.tile([128, T2], F32, tag="pcv")
                        nc.tensor.matmul(pcv[:], qw3_sb[:, o * 128:(o + 1) * 128],
                                         qe2[:, n * T2:(n + 1) * T2],
                                         start=True, stop=True)
                        nc.vector.tensor_scalar(qeT[o][:, n * T2:(n + 1) * T2],
                                                pcv[:], qb3_sb[:, o:o + 1],
                                                None, OP.add)

                # ================= prior loads (2 halves of 4 tiles) =================
                prh = {}
                for h in range(2):
                    for j in range(4):
                        prt = prpool.tile([128, T1 // 2], F32, tag="prt")
                        nc.sync.dma_start(
                            out=prt[:],
                            in_=d_pr[b, j * 128:(j + 1) * 128,
                                     h * (T1 // 2):(h + 1) * (T1 // 2)])
                        prh[(h, j)] = prt

                # ================= scores =================
                for i in range(NT1):
                    h, hoff = (0, 0) if i < 8 else (1, T1 // 2)
                    iq = i % 4
                    # z = 2T*qk - T*k2
                    pz = ps_z.tile([128, T2], F32, tag="pz")
                    nc.tensor.matmul(pz[:], qeT[0][:, i * 128:(i + 1) * 128],
                                     keT[0][:], start=True, stop=False)
                    nc.tensor.matmul(pz[:], qeT[1][:, i * 128:(i + 1) * 128],
                                     keT[1][:], start=False, stop=False)
                    nc.tensor.matmul(pz[:], ones_row[:], c2row[:],
                                     start=False, stop=True)
                    # priorT block via 4 PE transposes
                    ppm = ps_pm.tile([128, T2], F32, tag="ppm")
                    for j in range(4):
                        nc.tensor.transpose(
                            ppm[:, j * 128:(j + 1) * 128],
                            prh[(h, j)][:, i * 128 - hoff:(i + 1) * 128 - hoff],
                            ident_f[:])
                    logP = spool.tile([128, T2], F32, tag="logP")
                    nc.scalar.activation(logP[:], ppm[:], AF.Ln, bias=eps_col[:])
                    # lse = Ln(sum(Exp(z))): no max-subtraction needed, |z| tiny
                    sum1 = smallp.tile([128, 1], F32, tag="sum1")
                    e1 = spool.tile([128, T2], BF16, tag="e1")
                    nc.scalar.activation(e1[:], pz[:], AF.Exp, accum_out=sum1[:])
                    lse = smallp.tile([128, 1], F32, tag="lse")
                    nc.scalar.activation(lse[:], sum1[:], AF.Ln)

                    # stage buffers for 4 tiles then one 1MB DMA out
                    if iq == 0:
                        lp4 = stgpool.tile([128, 4, T2], F32, tag="lp4")
                        at4 = stgpool.tile([128, 4, T2], F32, tag="at4")
                    nc.vector.scalar_tensor_tensor(
                        lp4[:, iq, :], pz[:], lse[:], logP[:],
                        OP.subtract, OP.add)
                    e2 = spool.tile([128, T2], BF16, tag="e2")
                    nc.scalar.activation(e2[:], lp4[:, iq, :], AF.Exp)
                    e2m = spool.tile([128, T2], BF16, tag="e2m")
                    sum2 = smallp.tile([128, 1], F32, tag="sum2")
                    nc.vector.scalar_tensor_tensor(
                        e2m[:], e2[:], 1.0, m01rep[:],
                        OP.mult, OP.mult, accum_out=sum2[:])
                    r2 = smallp.tile([128, 1], F32, tag="r2")
                    nc.vector.reciprocal(r2[:], sum2[:])
                    nc.gpsimd.tensor_scalar(at4[:, iq, :], e2m[:], r2[:],
                                            None, OP.mult)
                    if iq == 3:
                        i0 = i - 3
                        nc.sync.dma_start(
                            out=d_lp[b, 0, i0 * 128:(i0 + 4) * 128, :]
                            .rearrange("(g p) t -> p g t", p=128),
                            in_=lp4[:])
                        nc.sync.dma_start(
                            out=d_attn[b, 0, i0 * 128:(i0 + 4) * 128, :]
                            .rearrange("(g p) t -> p g t", p=128),
                            in_=at4[:])

        if repeat == 1:
            emit(0)
        else:
            with tc.For_i(0, repeat, 1):
                emit(0)


_CACHE = {}


def _get_nc(repeat: int = 1):
    if repeat not in _CACHE:
        _CACHE[repeat] = build_nc(repeat)
    return _CACHE[repeat]


def make_in_maps(queries, keys, mask, attn_prior,
                 kw1, kb1, kw2, kb2, qw1, qb1, qw2, qb2, qw3, qb3):
    queries = np.ascontiguousarray(queries, dtype=np.float32)
    keys = np.ascontiguousarray(keys, dtype=np.float32)
    attn_prior = np.ascontiguousarray(attn_prior, dtype=np.float32)
    m01 = np.ascontiguousarray(1.0 - np.asarray(mask, dtype=np.float32))
    w = dict(
        kw1=np.ascontiguousarray(kw1, dtype=np.float32),
        kb1=np.ascontiguousarray(kb1, dtype=np.float32),
        kw2=np.ascontiguousarray(np.asarray(kw2, dtype=np.float32).reshape(2 * N_TEXT, N_ATT)),
        kb2=np.ascontiguousarray(kb2, dtype=np.float32),
        qw1=np.ascontiguousarray(qw1, dtype=np.float32),
        qb1=np.ascontiguousarray(qb1, dtype=np.float32),
        qw2=np.ascontiguousarray(np.asarray(qw2, dtype=np.float32).reshape(2 * N_MEL, N_MEL)),
        qb2=np.ascontiguousarray(qb2, dtype=np.float32),
        qw3=np.ascontiguousarray(np.asarray(qw3, dtype=np.float32).reshape(N_MEL, N_ATT)),
        qb3=np.ascontiguousarray(qb3, dtype=np.float32),
    )
    in_maps = []
    for c in range(NCORES):
        s = slice(c * PB, (c + 1) * PB)
        in_maps.append(dict(
            queries=queries[s], keys=keys[s], m01row=m01[s], prior=attn_prior[s],
            **w))
    return in_maps


def kernel(queries, keys, mask, attn_prior,
           kw1, kb1, kw2, kb2, qw1, qb1, qw2, qb2, qw3, qb3):
    from concourse import bass_utils
    nc = _get_nc(1)
    in_maps = make_in_maps(queries, keys, mask, attn_prior,
                           kw1, kb1, kw2, kb2, qw1, qb1, qw2, qb2, qw3, qb3)
    res = bass_utils.run_bass_kernel_spmd(nc, in_maps, core_ids=list(range(NCORES)))
    attn = np.concatenate([res.results[c]["attn"] for c in range(NCORES)], axis=0)
    lp = np.concatenate([res.results[c]["attn_logprob"] for c in range(NCORES)], axis=0)
    return attn, lp


# revision 32
# speedup vs baseline: 2.0197x; 1.4049x over previous
"""Trainium2 Bass kernel for nn_AlignmentEncoder.

Data-parallel over batch: 16 batches -> 8 cores x 2 batches.

Per core, per batch b:
  key path:   keys (512,256) -> keysT (256,512) via PE transpose ->
              conv k3 256->512 (PE) + relu -> conv k1 512->256 (PE) -> keT
              k2 = sum_c keT^2 (ACT square is avoided; DVE mult + PE ones-reduce)
              c2row = -TEMP * k2  (per-t2 row vector)
  query path: queries (80,2048) loads naturally as qT; conv chain on PE,
              epilogues (bias+relu) on DVE; qw3/qb3 pre-scaled by 2*TEMP so
              z = 2T*qk - T*k2 comes straight out of PSUM.
  scores, per t1-tile i (16 of 128 rows):
              z psum = qeT_i^T keT + ones x c2row           (PE)
              pm psum = priorT block (4x PE transpose)
              logP = Ln(pm + 1e-8)                          (ACT, f32)
              e1, sum1 = Exp(z), accum                      (ACT)
              lse = Ln(sum1)  [batched over 4 tiles]        (ACT)
              lp = (z - lse) + logP  -> attn_logprob out    (DVE stt, f32)
              e2 = Exp(lp)                                  (ACT, bf16)
              e2m, sum2 = e2 * m01rep, accum                (DVE ttr)
              attn = e2m * (1/sum2) -> attn out             (GPSIMD ts, f32)

log_softmax identities used: q2 term cancels entirely; no max-subtraction is
needed because z = 2T*qk - T*k2 is confined to a tiny range (|z| < ~1).
"""

import numpy as np

import concourse.bass as bass
import concourse.tile as tile
from concourse import bacc, mybir
from concourse.masks import make_identity

F32 = mybir.dt.float32
BF16 = mybir.dt.bfloat16
AF = mybir.ActivationFunctionType
OP = mybir.AluOpType

B, T1, T2 = 16, 2048, 512
N_MEL, N_TEXT, N_ATT = 80, 256, 256
TEMP = 0.0005
NCORES = 8
PB = B // NCORES  # batches per core
NT1 = T1 // 128   # t1 tiles per batch
EPS = 1e-8


def build_nc(repeat: int = 1, score_tiles: int = NT1, loop_only: bool = False):
    nc = bacc.Bacc("TRN2", target_bir_lowering=False, debug=False,
                   enable_asserts=False)

    # ---- per-core DRAM I/O ----
    d_q = nc.dram_tensor("queries", [PB, N_MEL, T1], F32, kind="ExternalInput").ap()
    d_k = nc.dram_tensor("keys", [PB, T2, N_TEXT], F32, kind="ExternalInput").ap()
    d_m01 = nc.dram_tensor("m01row", [PB, T2], F32, kind="ExternalInput").ap()
    d_pr = nc.dram_tensor("prior", [PB, T2, T1], F32, kind="ExternalInput").ap()
    d_kw1 = nc.dram_tensor("kw1", [3, N_TEXT, 2 * N_TEXT], F32, kind="ExternalInput").ap()
    d_kb1 = nc.dram_tensor("kb1", [2 * N_TEXT], F32, kind="ExternalInput").ap()
    d_kw2 = nc.dram_tensor("kw2", [2 * N_TEXT, N_ATT], F32, kind="ExternalInput").ap()
    d_kb2 = nc.dram_tensor("kb2", [N_ATT], F32, kind="ExternalInput").ap()
    d_qw1 = nc.dram_tensor("qw1", [3, N_MEL, 2 * N_MEL], F32, kind="ExternalInput").ap()
    d_qb1 = nc.dram_tensor("qb1", [2 * N_MEL], F32, kind="ExternalInput").ap()
    d_qw2 = nc.dram_tensor("qw2", [2 * N_MEL, N_MEL], F32, kind="ExternalInput").ap()
    d_qb2 = nc.dram_tensor("qb2", [N_MEL], F32, kind="ExternalInput").ap()
    d_qw3 = nc.dram_tensor("qw3", [N_MEL, N_ATT], F32, kind="ExternalInput").ap()
    d_qb3 = nc.dram_tensor("qb3", [N_ATT], F32, kind="ExternalInput").ap()
    d_attn = nc.dram_tensor("attn", [PB, 1, T1, T2], BF16, kind="ExternalOutput").ap()
    d_lp = nc.dram_tensor("attn_logprob", [PB, 1, T1, T2], F32, kind="ExternalOutput").ap()

    with tile.TileContext(nc) as tc:
        if loop_only:
            with tc.tile_pool(name="tiny", bufs=1) as tiny:
                def ebody():
                    t = tiny.tile([128, 128], F32, tag="t", name="t")
                    nc.gpsimd.memset(t[:, 0:1], 0.0)
                    nc.sync.dma_start(out=d_attn[0, 0, 0:128, 0:128], in_=t[:])
                if repeat == 1:
                    ebody()
                else:
                    with tc.For_i(0, repeat, 1):
                        ebody()
        else:
            _body(tc, repeat, score_tiles,
                  d_q, d_k, d_m01, d_pr,
                  d_kw1, d_kb1, d_kw2, d_kb2,
                  d_qw1, d_qb1, d_qw2, d_qb2, d_qw3, d_qb3,
                  d_attn, d_lp)
    nc.compile()
    return nc


def _body(tc, repeat, score_tiles, d_q, d_k, d_m01, d_pr, d_kw1, d_kb1, d_kw2, d_kb2,
          d_qw1, d_qb1, d_qw2, d_qb2, d_qw3, d_qb3, d_attn, d_lp):
    nc = tc.nc
    from contextlib import ExitStack
    ctx = ExitStack()
    with ctx:
        const = ctx.enter_context(tc.tile_pool(name="const", bufs=1))
        wpool = ctx.enter_context(tc.tile_pool(name="wpool", bufs=1))
        kpool = ctx.enter_context(tc.tile_pool(name="kpool", bufs=2))
        qpool = ctx.enter_context(tc.tile_pool(name="qpool", bufs=1))
        qepool = ctx.enter_context(tc.tile_pool(name="qepool", bufs=2))
        prpool = ctx.enter_context(tc.tile_pool(name="prpool", bufs=8))
        spool = ctx.enter_context(tc.tile_pool(name="spool", bufs=6))
        smallp = ctx.enter_context(tc.tile_pool(name="smallp", bufs=8))
        stgpool = ctx.enter_context(tc.tile_pool(name="stgpool", bufs=2))
        lpppool = ctx.enter_context(tc.tile_pool(name="lpppool", bufs=3))
        prtp = ctx.enter_context(tc.tile_pool(name="prtp", bufs=2))
        ps_z = ctx.enter_context(tc.tile_pool(name="ps_z", bufs=3, space="PSUM"))
        ps_cv = ctx.enter_context(tc.tile_pool(name="ps_cv", bufs=3, space="PSUM"))
        # all small PSUM tensors share one 2-slot tag (each <= 1 bank)
        ps_sm = ctx.enter_context(tc.tile_pool(name="ps_sm", bufs=2, space="PSUM"))

        def emit(it):
            # ---- constants ----
            ident_b = const.tile([128, 128], BF16, name=f"ident_b{it}")
            nc.vector.memset(ident_b[:], 0.0)
            nc.gpsimd.affine_select(
                out=ident_b[:], in_=ident_b[:],
                compare_op=OP.not_equal, fill=1.0, base=0,
                pattern=[[-1, 128]], channel_multiplier=1)
            ones_row = const.tile([1, 128], BF16, name=f"ones_row{it}")
            nc.vector.memset(ones_row[:], 1.0)
            ones_col = const.tile([128, 1], BF16, name=f"ones_col{it}")
            nc.vector.memset(ones_col[:], 1.0)
            eps_col = const.tile([128, 1], F32, name=f"eps_col{it}")
            nc.vector.memset(eps_col[:], EPS)

            # ---- weights (cast to bf16 during DMA on the SWDGE path) ----
            kw1_sb = wpool.tile([128, 3, 2, 2 * N_TEXT], BF16, name=f"kw1_sb{it}")
            nc.gpsimd.dma_start(
                out=kw1_sb[:],
                in_=d_kw1.rearrange("dt (ci p) o -> p dt ci o", p=128))
            kw2_sb = wpool.tile([128, 4, N_ATT], BF16, name=f"kw2_sb{it}")
            nc.gpsimd.dma_start(
                out=kw2_sb[:],
                in_=d_kw2.rearrange("(ci p) o -> p ci o", p=128))
            qw1_sb = wpool.tile([N_MEL, 3, 2 * N_MEL], BF16, name=f"qw1_sb{it}")
            nc.gpsimd.dma_start(
                out=qw1_sb[:], in_=d_qw1.rearrange("dt ci o -> ci dt o"))
            qw2a_sb = wpool.tile([128, N_MEL], BF16, name=f"qw2a_sb{it}")
            nc.gpsimd.dma_start(out=qw2a_sb[:], in_=d_qw2[0:128, :])
            qw2b_sb = wpool.tile([32, N_MEL], BF16, name=f"qw2b_sb{it}")
            nc.gpsimd.dma_start(out=qw2b_sb[:], in_=d_qw2[128:160, :])
            qw3_f = wpool.tile([N_MEL, N_ATT], F32, name=f"qw3_f{it}")
            nc.sync.dma_start(out=qw3_f[:], in_=d_qw3[:])
            qw3_sb = wpool.tile([N_MEL, N_ATT], BF16, name=f"qw3_sb{it}")
            nc.vector.tensor_scalar_mul(qw3_sb[:], qw3_f[:], 2.0 * TEMP)

            # biases as [128, ncols] column stacks
            kb1_sb = wpool.tile([128, 4], F32, name=f"kb1_sb{it}")
            nc.sync.dma_start(out=kb1_sb[:], in_=d_kb1.rearrange("(j p) -> p j", p=128))
            kb2_sb = wpool.tile([128, 2], F32, name=f"kb2_sb{it}")
            nc.sync.dma_start(out=kb2_sb[:], in_=d_kb2.rearrange("(j p) -> p j", p=128))
            qb1_sb = wpool.tile([128, 2], F32, name=f"qb1_sb{it}")
            nc.vector.memset(qb1_sb[:], 0.0)
            nc.sync.dma_start(out=qb1_sb[0:128, 0:1], in_=d_qb1[0:128].rearrange("(p o) -> p o", o=1))
            nc.sync.dma_start(out=qb1_sb[0:32, 1:2], in_=d_qb1[128:160].rearrange("(p o) -> p o", o=1))
            qb2_sb = wpool.tile([N_MEL, 1], F32, name=f"qb2_sb{it}")
            nc.sync.dma_start(out=qb2_sb[:], in_=d_qb2.rearrange("(p o) -> p o", o=1))
            qb3_f = wpool.tile([128, 2], F32, name=f"qb3_f{it}")
            nc.sync.dma_start(out=qb3_f[:], in_=d_qb3.rearrange("(j p) -> p j", p=128))
            qb3_sb = wpool.tile([128, 2], F32, name=f"qb3_sb{it}")
            nc.vector.tensor_scalar_mul(qb3_sb[:], qb3_f[:], 2.0 * TEMP)

            pend = []

            def phase_a(g, b, qeT, keT, c2row, prT):
                sum1s = smallp.tile([128, 4], F32, tag="sum1s", name="sum1s")
                lpp4 = lpppool.tile([128, 4, T2], F32, tag="lpp4", name="lpp4")
                for k in range(4):
                    i = 4 * g + k
                    pz = ps_z.tile([128, T2], F32, tag="pz", name="pz")
                    nc.tensor.matmul(pz[:], qeT[0][:, i * 128:(i + 1) * 128],
                                     keT[0][:], start=True, stop=False)
                    nc.tensor.matmul(pz[:], qeT[1][:, i * 128:(i + 1) * 128],
                                     keT[1][:], start=False, stop=False)
                    nc.tensor.matmul(pz[:], ones_row[:], c2row[:],
                                     start=False, stop=True)
                    logP = spool.tile([128, T2], F32, tag="logP", name="logP")
                    nc.scalar.activation(logP[:], prT[i // 8][:, i % 8, :, :],
                                         AF.Ln, bias=eps_col[:])
                    e1 = spool.tile([128, T2], BF16, tag="e1", name="e1")
                    nc.scalar.activation(e1[:], pz[:], AF.Exp,
                                         accum_out=sum1s[:, k:k + 1])
                    nc.vector.tensor_add(lpp4[:, k, :], pz[:], logP[:])
                return sum1s, lpp4

            def phase_b_early(sum1s, lpp4, g, b, m01rep):
                lses = smallp.tile([128, 4], F32, tag="lses", name="lses")
                nc.scalar.activation(lses[:], sum1s[:], AF.Ln)
                lp4 = stgpool.tile([128, 4, T2], F32, tag="lp4", name="lp4")
                for k in range(4):
                    nc.vector.tensor_scalar(lp4[:, k, :], lpp4[:, k, :],
                                            lses[:, k:k + 1], None, OP.subtract)
                return lp4

            def phase_b_late(lp4, g, b, m01rep):
                at4 = stgpool.tile([128, 4, T2], BF16, tag="at4", name="at4")
                for k in range(4):
                    e2 = spool.tile([128, T2], BF16, tag="e2", name="e2")
                    nc.scalar.activation(e2[:], lp4[:, k, :], AF.Exp)
                    e2m = spool.tile([128, T2], BF16, tag="e2m", name="e2m")
                    sum2 = smallp.tile([128, 1], F32, tag="sum2", name="sum2")
                    nc.vector.scalar_tensor_tensor(
                        e2m[:], e2[:], 1.0, m01rep[:],
                        OP.mult, OP.mult, accum_out=sum2[:])
                    r2 = smallp.tile([128, 1], F32, tag="r2", name="r2")
                    nc.vector.reciprocal(r2[:], sum2[:])
                    nc.vector.tensor_scalar(at4[:, k, :], e2m[:], r2[:],
                                            None, OP.mult)
                i0 = 4 * g
                nc.sync.dma_start(
                    out=d_lp[b, 0, i0 * 128:(i0 + 4) * 128, :]
                    .rearrange("(g p) t -> p g t", p=128), in_=lp4[:])
                nc.sync.dma_start(
                    out=d_attn[b, 0, i0 * 128:(i0 + 4) * 128, :]
                    .rearrange("(g p) t -> p g t", p=128), in_=at4[:])

            for b in range(PB):
                # ================= key path =================
                keys_nat = kpool.tile([128, 4, N_TEXT], BF16, tag="keys_nat")
                nc.gpsimd.dma_start(
                    out=keys_nat[:],
                    in_=d_k[b].rearrange("(j p) c -> p j c", p=128))
                # keysT: [c, t2] with zero-padded t2 edges, 2 c-tiles
                keysT = [kpool.tile([128, T2 + 2], BF16, tag=f"keysT{ci}", name=f"keysT{ci}")
                         for ci in range(2)]
                for ci in range(2):
                    nc.vector.memset(keysT[ci][:, 0:1], 0.0)
                    nc.vector.memset(keysT[ci][:, T2 + 1:T2 + 2], 0.0)
                for ci in range(2):
                    pst = ps_cv.tile([128, T2], BF16, tag="pcv", name="pst")
                    for j in range(4):
                        nc.tensor.transpose(pst[:, j * 128:(j + 1) * 128],
                                            keys_nat[:, j, ci * 128:(ci + 1) * 128],
                                            ident_b[:])
                    nc.vector.tensor_copy(keysT[ci][:, 1:T2 + 1], pst[:])
                # kconv1 (k=3, 256->512) + relu
                ke1T = [kpool.tile([128, T2], BF16, tag=f"ke1T{j}", name=f"ke1T{j}") for j in range(4)]
                for j in range(4):
                    pcv = ps_cv.tile([128, T2], F32, tag="pcv")
                    first = True
                    for dt in range(3):
                        for ci in range(2):
                            nc.tensor.matmul(
                                pcv[:], kw1_sb[:, dt, ci, j * 128:(j + 1) * 128],
                                keysT[ci][:, dt:dt + T2],
                                start=first, stop=(dt == 2 and ci == 1))
                            first = False
                    nc.scalar.activation(ke1T[j][:], pcv[:], AF.Relu,
                                         bias=kb1_sb[:, j:j + 1])
                # kconv2 (k=1, 512->256)
                keT = [kpool.tile([128, T2], BF16, tag=f"keT{j2}", name=f"keT{j2}") for j2 in range(2)]
                for j2 in range(2):
                    pcv = ps_cv.tile([128, T2], F32, tag="pcv")
                    for ci1 in range(4):
                        nc.tensor.matmul(pcv[:], kw2_sb[:, ci1, j2 * 128:(j2 + 1) * 128],
                                         ke1T[ci1][:],
                                         start=(ci1 == 0), stop=(ci1 == 3))
                    nc.scalar.activation(keT[j2][:], pcv[:], AF.Identity,
                                         bias=kb2_sb[:, j2:j2 + 1])
                # k2 = sum_c keT^2 ; c2row = -TEMP * k2
                sqk = [kpool.tile([128, T2], BF16, tag=f"sqk{j2}", name=f"sqk{j2}") for j2 in range(2)]
                for j2 in range(2):
                    nc.vector.tensor_mul(sqk[j2][:], keT[j2][:], keT[j2][:])
                pk2 = ps_sm.tile([1, T2], F32, tag="sm", name="pk2")
                for j2 in range(2):
                    nc.tensor.matmul(pk2[:], ones_col[:], sqk[j2][:],
                                     start=(j2 == 0), stop=(j2 == 1))
                c2row = smallp.tile([1, T2], BF16, tag="c2row")
                nc.scalar.activation(c2row[:], pk2[:], AF.Copy, scale=-TEMP)

                # m01rep: [128, T2] bf16 broadcast of the valid-mask row
                m01_b = smallp.tile([1, T2], BF16, tag="m01_b")
                nc.gpsimd.dma_start(out=m01_b[:], in_=d_m01[b].rearrange("(o t) -> o t", o=1))
                pmr = ps_sm.tile([128, T2], F32, tag="sm", name="pmr")
                nc.tensor.matmul(pmr[:], ones_row[:], m01_b[:], start=True, stop=True)
                m01rep = kpool.tile([128, T2], BF16, tag="m01rep")
                nc.scalar.activation(m01rep[:], pmr[:], AF.Copy)

                # ================= query path =================
                qT = qpool.tile([N_MEL, T1 + 2], BF16, tag="qT")
                nc.vector.memset(qT[:, 0:1], 0.0)
                nc.vector.memset(qT[:, T1 + 1:T1 + 2], 0.0)
                nc.gpsimd.dma_start(out=qT[:, 1:T1 + 1], in_=d_q[b])
                # qconv1 (k=3, 80->160) + relu: o-tiles [128, 32]
                qe1a = qpool.tile([128, T1], BF16, tag="qe1a")
                qe1b = qpool.tile([32, T1], BF16, tag="qe1b")
                for n in range(4):
                    for (oi, (qe1, o0, ow)) in enumerate(
                            [(qe1a, 0, 128), (qe1b, 128, 32)]):
                        pcv = ps_cv.tile([128, T2], F32, tag="pcv")
                        for dt in range(3):
                            nc.tensor.matmul(
                                pcv[0:ow, :], qw1_sb[:, dt, o0:o0 + ow],
                                qT[:, dt + n * T2:dt + (n + 1) * T2],
                                start=(dt == 0), stop=(dt == 2))
                        nc.vector.tensor_scalar(
                            qe1[:, n * T2:(n + 1) * T2], pcv[0:ow, :],
                            qb1_sb[0:ow, oi:oi + 1], 0.0, OP.add, OP.max)
                # qconv2 (k=1, 160->80) + relu
                qe2 = qpool.tile([N_MEL, T1], BF16, tag="qe2")
                for n in range(4):
                    pcv = ps_cv.tile([128, T2], F32, tag="pcv")
                    nc.tensor.matmul(pcv[0:N_MEL, :], qw2a_sb[:],
                                     qe1a[:, n * T2:(n + 1) * T2],
                                     start=True, stop=False)
                    nc.tensor.matmul(pcv[0:N_MEL, :], qw2b_sb[:],
                                     qe1b[:, n * T2:(n + 1) * T2],
                                     start=False, stop=True)
                    nc.vector.tensor_scalar(qe2[:, n * T2:(n + 1) * T2],
                                            pcv[0:N_MEL, :], qb2_sb[:],
                                            0.0, OP.add, OP.max)
                # qconv3 (k=1, 80->256), scaled by 2*TEMP
                qeT = [qepool.tile([128, T1], BF16, tag=f"qeT{o}", name=f"qeT{o}") for o in range(2)]
                for o in range(2):
                    for n in range(4):
                        pcv = ps_cv.tile([128, T2], F32, tag="pcv")
                        nc.tensor.matmul(pcv[:], qw3_sb[:, o * 128:(o + 1) * 128],
                                         qe2[:, n * T2:(n + 1) * T2],
                                         start=True, stop=True)
                        nc.vector.tensor_scalar(qeT[o][:, n * T2:(n + 1) * T2],
                                                pcv[:], qb3_sb[:, o:o + 1],
                                                None, OP.add)

                # ===== prior: cast-load bf16 then xbar-transpose to [t1, t2] =====
                prT = []
                for h in range(2):
                    prTh = prtp.tile([128, 8, 4, 128], BF16, tag="prTh", name="prTh")
                    for j in range(4):
                        prt = prpool.tile([128, T1 // 2], BF16, tag="prt", name="prt")
                        nc.gpsimd.dma_start(
                            out=prt[:],
                            in_=d_pr[b, j * 128:(j + 1) * 128,
                                     h * (T1 // 2):(h + 1) * (T1 // 2)])
                        nc.sync.dma_start_transpose(out=prTh[:, :, j, :], in_=prt[:])
                    prT.append(prTh)

                # ================= scores =================
                # software-pipelined in groups of 4 t1-tiles: phase A does
                # PE + Ln(prior) + exp-accum + lpp = z + logP (frees PSUM);
                # phase B (one group behind) does batched lse, the two
                # outputs, and the store DMAs.  The 1-group offset keeps each
                # engine's static instruction order free of head-of-line
                # stalls on cross-engine dependencies.
                assert score_tiles % 4 == 0
                for g in range(score_tiles // 4):
                    late_args = None
                    if len(pend) >= 2:
                        sum1s_p, lpp4_p, g_p, b_p, m01rep_p = pend.pop(0)
                        lp4_p = phase_b_early(sum1s_p, lpp4_p, g_p, b_p, m01rep_p)
                        late_args = (lp4_p, g_p, b_p, m01rep_p)
                    a_state = phase_a(g, b, qeT, keT, c2row, prT)
                    if late_args is not None:
                        phase_b_late(*late_args)
                    pend.append((*a_state, g, b, m01rep))
            if b == PB - 1:
                while pend:
                    sum1s_p, lpp4_p, g_p, b_p, m01rep_p = pend.pop(0)
                    lp4_p = phase_b_early(sum1s_p, lpp4_p, g_p, b_p, m01rep_p)
                    phase_b_late(lp4_p, g_p, b_p, m01rep_p)

        if repeat == 1:
            emit(0)
        else:
            with tc.For_i(0, repeat, 1):
                emit(0)


_CACHE = {}


def _get_nc(repeat: int = 1, score_tiles: int = NT1, loop_only: bool = False):
    key = (repeat, score_tiles, loop_only)
    if key not in _CACHE:
        _CACHE[key] = build_nc(repeat, score_tiles, loop_only)
    return _CACHE[key]


def make_in_maps(queries, keys, mask, attn_prior,
                 kw1, kb1, kw2, kb2, qw1, qb1, qw2, qb2, qw3, qb3):
    queries = np.ascontiguousarray(queries, dtype=np.float32)
    keys = np.ascontiguousarray(keys, dtype=np.float32)
    attn_prior = np.ascontiguousarray(attn_prior, dtype=np.float32)
    m01 = np.ascontiguousarray(1.0 - np.asarray(mask, dtype=np.float32))
    w = dict(
        kw1=np.ascontiguousarray(kw1, dtype=np.float32),
        kb1=np.ascontiguousarray(kb1, dtype=np.float32),
        kw2=np.ascontiguousarray(np.asarray(kw2, dtype=np.float32).reshape(2 * N_TEXT, N_ATT)),
        kb2=np.ascontiguousarray(kb2, dtype=np.float32),
        qw1=np.ascontiguousarray(qw1, dtype=np.float32),
        qb1=np.ascontiguousarray(qb1, dtype=np.float32),
        qw2=np.ascontiguousarray(np.asarray(qw2, dtype=np.float32).reshape(2 * N_MEL, N_MEL)),
        qb2=np.ascontiguousarray(qb2, dtype=np.float32),
        qw3=np.ascontiguousarray(np.asarray(qw3, dtype=np.float32).reshape(N_MEL, N_ATT)),
        qb3=np.ascontiguousarray(qb3, dtype=np.float32),
    )
    in_maps = []
    for c in range(NCORES):
        s = slice(c * PB, (c + 1) * PB)
        in_maps.append(dict(
            queries=queries[s], keys=keys[s], m01row=m01[s], prior=attn_prior[s],
            **w))
    return in_maps


def kernel(queries, keys, mask, attn_prior,
           kw1, kb1, kw2, kb2, qw1, qb1, qw2, qb2, qw3, qb3):
    from concourse import bass_utils
    nc = _get_nc(1)
    in_maps = make_in_maps(queries, keys, mask, attn_prior,
                           kw1, kb1, kw2, kb2, qw1, qb1, qw2, qb2, qw3, qb3)
    res = bass_utils.run_bass_kernel_spmd(nc, in_maps, core_ids=list(range(NCORES)))
    attn = np.concatenate([res.results[c]["attn"].astype(np.float32)
                           for c in range(NCORES)], axis=0)
    lp = np.concatenate([res.results[c]["attn_logprob"] for c in range(NCORES)], axis=0)
    return attn, lp
